# revision 17
# baseline (speedup 1.0000x reference)
"""Trainium2 Bass kernel for nn_MultiHeadAttention_2250562863251.

Key algebraic insight: the reference einsum 'mbhi,nbhj->mnbh' contracts i and j
independently, so scores[m,n,b,h] = (sum_i q[m,b,h,i]) * (sum_j k[n,b,h,j]) —
a rank-1 outer product of per-head row-sums. Full Q/K projections are never
needed; only queries @ (per-head-summed Wq) [E,H], computed on host (tiny).

Sharding: 8 cores = 2 (batch) x 4 (head-groups of 4 heads).

Device per core (batch bi, heads hg*4..hg*4+3):
  - v-proj:  v = values_b @ WvL.T   (PE, fp32r, via host-transposed valuesT)
  - scores:  scT[n,m] = qs_bcast[m]*c'_n + beta_bcast[m]   (DVE stt, fp32)
             + tri additive causal mask on diagonal blocks
  - exp:     eT = exp(scT + d_n)    (ACT, per-partition bias, bf16 out)
    where beta_m = -(qs_m * suffix-extreme(c)) = -rowmax (host), so the
    softmax max-subtraction is folded into the score build, and d_n = -1000
    padding mask folds into the ACT bias.
  - attn:    pooled[m, 65] accumulates eT.T @ [v_h | ones] over n-chunks
             (bf16 matmul; ones column yields the softmax denominator)
  - divide:  pooled[:, :64] * recip(pooled[:, 64])  per head
  - out-proj: outT[o, m] = WoLT.T @ pooledT (PE transpose + fp32r matmul)
Host assembles: out[m,b,:] = sum_hg outT.T + bo + bv @ Wo.T, with exact
recompute of (rare/absent) degenerate rows where rowmax <= -990.
"""
import sys

for _p in ("/opt/trn_rl_repo", "/root/.axon_site/_ro/trn_rl_repo"):
    if _p not in sys.path:
        sys.path.append(_p)

import numpy as np
import ml_dtypes

import concourse.bass as bass
import concourse.mybir as mybir
import concourse.tile as tile
from concourse import bacc
from concourse.bass_utils import run_bass_kernel_spmd
from concourse.masks import make_identity

# Problem shapes (hardcoded per contract)
M = 2048   # query positions
N = 2048   # key positions
B = 2
E = 1024
H = 16
DH = 64        # head dim
HL = 4         # heads per core
KL = HL * DH   # 256 local pooled dims
NEG = -1000.0
P = 128
NK = N // P    # 16 n-chunks
T = 4          # m-tiles of 512
MT = 512
NCORES = 8

f32 = mybir.dt.float32
f32r = mybir.dt.float32r
bf16 = mybir.dt.bfloat16

_CACHE = {}


def _build_program():
    if "nc" in _CACHE:
        return _CACHE["nc"]
    nc = bacc.Bacc("TRN2", target_bir_lowering=False, debug=False,
                   num_devices=NCORES)

    vt_d = nc.declare_dram_parameter("vt", [E, N], bf16, isOutput=False)
    wvlt_d = nc.declare_dram_parameter("wvlt", [E, KL], bf16, isOutput=False)
    wolt_d = nc.declare_dram_parameter("wolt", [KL, E], bf16, isOutput=False)
    qsl_d = nc.declare_dram_parameter("qsl", [HL, M], f32, isOutput=False)
    betal_d = nc.declare_dram_parameter("betal", [HL, M], bf16, isOutput=False)
    cd_d = nc.declare_dram_parameter("cd", [N, HL + 1], f32, isOutput=False)
    tri_d = nc.declare_dram_parameter("tri", [4 * P, MT], f32, isOutput=False)
    outp_d = nc.declare_dram_parameter("outp", [E, M], f32, isOutput=True)
    if _CACHE.get("debug"):
        dbg = {
            "d_vsb": nc.declare_dram_parameter("d_vsb", [P, NK, HL * (DH + 1)], bf16, isOutput=True),
            "d_sc0": nc.declare_dram_parameter("d_sc0", [P, MT], f32, isOutput=True),
            "d_sc15": nc.declare_dram_parameter("d_sc15", [P, MT], f32, isOutput=True),
            "d_et0": nc.declare_dram_parameter("d_et0", [P, MT], bf16, isOutput=True),
            "d_et15": nc.declare_dram_parameter("d_et15", [P, MT], bf16, isOutput=True),
            "d_pool": nc.declare_dram_parameter("d_pool", [DH + 1, MT], f32, isOutput=True),
            "d_rec": nc.declare_dram_parameter("d_rec", [1, MT], f32, isOutput=True),
            "d_recb": nc.declare_dram_parameter("d_recb", [DH, MT], f32, isOutput=True),
            "d_ptn": nc.declare_dram_parameter("d_ptn", [DH, HL, MT], bf16, isOutput=True),
        }

    with tile.TileContext(nc) as tc:
        with (
            tc.tile_pool(name="const", bufs=1) as const,
            tc.tile_pool(name="vstream", bufs=2) as vstream,
            tc.tile_pool(name="bcast", bufs=2) as bcast,
            tc.tile_pool(name="work", bufs=3) as work,
            tc.tile_pool(name="et_pool", bufs=4) as et_pool,
            tc.tile_pool(name="ptn", bufs=4) as ptn,
            tc.tile_pool(name="small", bufs=4) as small,
            tc.tile_pool(name="opool", bufs=3) as opool,
            tc.tile_pool(name="dpool", bufs=4, space="DRAM") as dpool,
            tc.tile_pool(name="ps_v", bufs=2, space="PSUM") as ps_v,
            tc.tile_pool(name="ps_pool", bufs=2, space="PSUM") as ps_pool,
            tc.tile_pool(name="ps_o", bufs=2, space="PSUM") as ps_o,
        ):
            # ---- resident constants ----
            wvlt_sb = const.tile([P, E // P, KL], bf16)
            nc.sync.dma_start(wvlt_sb[:], wvlt_d.rearrange("(ek p) d -> p ek d", p=P))
            wolt_sb = const.tile([DH, HL, E], bf16)
            nc.sync.dma_start(wolt_sb[:], wolt_d.rearrange("(kb p) o -> p kb o", p=DH))
            cd_sb = const.tile([P, NK, HL + 1], f32)
            nc.sync.dma_start(cd_sb[:], cd_d.rearrange("(k p) f -> p k f", p=P))
            tri_sb = const.tile([P, 4, MT], f32)
            nc.sync.dma_start(tri_sb[:], tri_d.rearrange("(pos p) m -> p pos m", p=P))
            # v_sb[:, k, h*65 : h*65+64] = v for head h, chunk k; col h*65+64 = 1.0
            v_sb = const.tile([P, NK, HL * (DH + 1)], bf16)
            nc.vector.memset(v_sb[:], 1.0)

            # ---- stage 1: v projection ----
            for q in range(4):  # n-quarters of 512
                vt_sb = vstream.tile([P, E // P, MT], bf16)
                nc.sync.dma_start(
                    vt_sb[:],
                    vt_d[:, q * MT:(q + 1) * MT].rearrange("(ek p) n -> p ek n", p=P),
                )
                for nk_r in range(4):
                    k = q * 4 + nk_r
                    vps = ps_v.tile([P, KL], f32)
                    for ek in range(E // P):
                        nc.tensor.matmul(
                            vps[:],
                            vt_sb[:, ek, nk_r * P:(nk_r + 1) * P],
                            wvlt_sb[:, ek, :],
                            start=(ek == 0),
                            stop=(ek == E // P - 1),
                        )
                    nc.any.tensor_copy(
                        out=v_sb[:, k].rearrange("p (h x) -> p h x", x=DH + 1)[:, :, 0:DH],
                        in_=vps.rearrange("p (h x) -> p h x", x=DH),
                    )

            # ---- stage 2: scores / softmax / attention ----
            pTn_all = {}
            for t in range(T):
                qsb = bcast.tile([P, HL, MT], f32, tag="qsb")
                nc.sync.dma_start(
                    qsb[:],
                    qsl_d[None, :, t * MT:(t + 1) * MT].to_broadcast([P, HL, MT]),
                )
                bb = bcast.tile([P, HL, MT], bf16, tag="bb")
                nc.sync.dma_start(
                    bb[:],
                    betal_d[None, :, t * MT:(t + 1) * MT].to_broadcast([P, HL, MT]),
                )

                # pTn[:, h, :] = pooled_h^T / rowsum_h  (bf16, K=64 chunks)
                pTn = ptn.tile([DH, HL, MT], bf16)
                pTn_all[t] = pTn
                for h in range(HL):
                    # pooled^T accumulated over n-chunks: rows 0..63 = v-dims,
                    # row 64 = softmax denominator (ones column of v_sb)
                    pool_ps = ps_pool.tile([DH + 1, MT], f32)
                    for k in range(4 * t, NK):
                        sc = work.tile([P, MT], f32, tag="sc")
                        nc.vector.scalar_tensor_tensor(
                            out=sc[:],
                            in0=qsb[:, h],
                            scalar=cd_sb[:, k, h:h + 1],
                            in1=bb[:, h],
                            op0=mybir.AluOpType.mult,
                            op1=mybir.AluOpType.add,
                        )
                        pos = k - 4 * t
                        if pos < 4:
                            nc.vector.tensor_add(
                                out=sc[:], in0=sc[:], in1=tri_sb[:, pos]
                            )
                        et = et_pool.tile([P, MT], bf16, tag="et")
                        nc.scalar.activation(
                            et[:], sc[:], mybir.ActivationFunctionType.Exp,
                            bias=cd_sb[:, k, HL:HL + 1],
                        )
                        if _CACHE.get("debug") and t == 0 and h == 0 and k in (0, 15):
                            nc.sync.dma_start(dbg[f"d_sc{k}"][:], sc[:])
                            nc.sync.dma_start(dbg[f"d_et{k}"][:], et[:])
                        nc.tensor.matmul(
                            pool_ps[:],
                            v_sb[:, k, h * (DH + 1):(h + 1) * (DH + 1)],
                            et[:],
                            start=(k == 4 * t),
                            stop=(k == NK - 1),
                        )
                    if _CACHE.get("debug") and t == 0 and h == 0:
                        pool_dbg = work.tile([DH + 1, MT], f32, tag="pool_dbg")
                        nc.vector.tensor_copy(out=pool_dbg[:], in_=pool_ps[:])
                        nc.sync.dma_start(dbg["d_pool"][:], pool_dbg[:])
                    rec = small.tile([DH + 1, MT], f32, tag="rec")
                    nc.vector.reciprocal(rec[DH:DH + 1, :], pool_ps[DH:DH + 1, :])
                    recb = small.tile([DH, MT], f32, tag="recb")
                    rd = dpool.tile([1, MT], f32, tag="rd")
                    nc.sync.dma_start(rd[:], rec[DH:DH + 1, :])
                    nc.sync.dma_start(
                        recb[:], rd[0][None, :].to_broadcast([DH, MT])
                    )
                    nc.vector.tensor_mul(
                        out=pTn[:, h, :],
                        in0=pool_ps[0:DH, :],
                        in1=recb[:],
                    )
                    if _CACHE.get("debug") and t == 0 and h == 0:
                        nc.sync.dma_start(dbg["d_rec"][:], rec[DH:DH + 1, :])
                        nc.sync.dma_start(dbg["d_recb"][:], recb[:])
                    if _CACHE.get("debug") and t == 0 and h == HL - 1:
                        nc.sync.dma_start(dbg["d_ptn"][:], pTn[:])
                        nc.sync.dma_start(dbg["d_vsb"][:], v_sb[:])

            # ---- out-projection: outT[o, m] = sum_h WoLT_h.T @ pTn_h ----
            for ob in range(E // P):
                for t in range(T):
                    ops = ps_o.tile([P, MT], f32)
                    for kb in range(HL):
                        nc.tensor.matmul(
                            ops[:],
                            wolt_sb[0:DH, kb, ob * P:(ob + 1) * P],
                            pTn_all[t][:, kb, :],
                            start=(kb == 0),
                            stop=(kb == HL - 1),
                        )
                    osb = opool.tile([P, MT], f32, tag="osb")
                    if (ob + t) % 2 == 0:
                        nc.vector.tensor_copy(out=osb[:], in_=ops[:])
                    else:
                        nc.scalar.copy(osb[:], ops[:])
                    nc.sync.dma_start(
                        outp_d[ob * P:(ob + 1) * P, t * MT:(t + 1) * MT], osb[:]
                    )

    nc.compile()
    _CACHE["nc"] = nc
    return nc


def _host_prep(queries, keys, values, Wq, bq, Wk, bk, Wv, bv, Wo, bo, in_mask):
    """Host-side prep. Returns (in_maps, fixup, extras)."""
    qs = np.einsum("mbe,he->mbh", queries, Wq.reshape(H, DH, E).sum(1),
                   dtype=np.float32) + bq.reshape(H, DH).sum(1)
    ks = np.einsum("nbe,he->nbh", keys, Wk.reshape(H, DH, E).sum(1),
                   dtype=np.float32) + bk.reshape(H, DH).sum(1)

    mask3 = in_mask[:, :, None]
    cp = np.where(mask3, 0.0, ks).astype(np.float32)          # [n, b, H]
    d = np.where(in_mask, NEG, 0.0).astype(np.float32)        # [n, b]

    cmax = np.where(mask3, -np.inf, ks)
    cmax = np.maximum.accumulate(cmax[::-1], axis=0)[::-1]    # suffix max, n>=m
    cmin = np.where(mask3, np.inf, ks)
    cmin = np.minimum.accumulate(cmin[::-1], axis=0)[::-1]
    nonempty = np.maximum.accumulate((~in_mask)[::-1], axis=0)[::-1]  # [n, b]

    with np.errstate(invalid="ignore"):
        A = np.where(qs >= 0, qs * cmax, qs * cmin)           # [m, b, H]
    A = np.where(nonempty[:, :, None], A, -np.inf)
    fixup_rows = np.any(~(A > -990.0), axis=2)                # [m, b] (nan-safe)
    beta = np.where(np.isfinite(A), -A, 1e4)
    beta = np.where(np.any(~(A > -990.0), axis=2)[:, :, None], -1e4, beta)
    beta = beta.astype(np.float32)

    in_maps = []
    vt_by_b = [np.ascontiguousarray(values[:, bi, :].T).astype(ml_dtypes.bfloat16)
               for bi in range(B)]
    tri = np.zeros((4 * P, MT), np.float32)
    for pos in range(4):
        nr = np.arange(P)[:, None] + 128 * pos
        mr = np.arange(MT)[None, :]
        tri[pos * P:(pos + 1) * P] = np.where(nr < mr, -4000.0, 0.0)

    for c in range(NCORES):
        bi, hg = c // 4, c % 4
        lh = slice(hg * HL, (hg + 1) * HL)
        ds = slice(hg * KL, (hg + 1) * KL)
        in_maps.append({
            "vt": vt_by_b[bi],
            "wvlt": np.ascontiguousarray(Wv[ds, :].T).astype(ml_dtypes.bfloat16),
            "wolt": np.ascontiguousarray(Wo[:, ds].T).astype(ml_dtypes.bfloat16),
            "qsl": np.ascontiguousarray(qs[:, bi, lh].T),
            "betal": np.ascontiguousarray(beta[:, bi, lh].T).astype(ml_dtypes.bfloat16),
            "cd": np.ascontiguousarray(
                np.concatenate([cp[:, bi, lh], d[:, bi:bi + 1]], axis=1)),
            "tri": tri,
        })
    return in_maps, fixup_rows, (qs, ks)


def _fixup_row(out, m, bi, qs, ks, values, Wv, bv, Wo, bo, in_mask):
    """Exact numpy recompute of one output row (degenerate / extreme rows)."""
    pot = qs[m, bi, :][None, :] * ks[:, bi, :]                # [n, H]
    pot = np.where(in_mask[:, bi][:, None], NEG, pot)
    causal = np.arange(N) < m                                 # mask n < m
    pot = np.where(causal[:, None], NEG, pot)
    pot = pot - pot.max(axis=0, keepdims=True)
    w = np.exp(pot)
    w = w / w.sum(axis=0, keepdims=True)                      # [n, H]
    v = (values[:, bi, :] @ Wv.T + bv).reshape(N, H, DH)
    pooled = np.einsum("nh,nhd->hd", w, v).reshape(E)
    out[m, bi, :] = pooled @ Wo.T + bo


def kernel(queries, keys, values, Wq, bq, Wk, bk, Wv, bv, Wo, bo, in_mask,
           _trace=False):
    args = (queries, keys, values, Wq, bq, Wk, bk, Wv, bv, Wo, bo)
    args = tuple(np.asarray(a, np.float32) for a in args)
    in_mask = np.asarray(in_mask, bool)
    (queries, keys, values, Wq, bq, Wk, bk, Wv, bv, Wo, bo) = args

    nc = _build_program()
    in_maps, fixup_rows, (qs, ks) = _host_prep(
        queries, keys, values, Wq, bq, Wk, bk, Wv, bv, Wo, bo, in_mask)

    res = run_bass_kernel_spmd(nc, in_maps, list(range(NCORES)), trace=_trace)
    results = res.results

    out = np.zeros((M, B, E), np.float32)
    for c in range(NCORES):
        bi = c // 4
        out[:, bi, :] += np.asarray(results[c]["outp"], np.float32).T
    out += (bo + bv @ Wo.T)[None, None, :]

    for m, bi in zip(*np.nonzero(fixup_rows)):
        _fixup_row(out, m, bi, qs, ks, values, Wv, bv, Wo, bo, in_mask)

    if _trace:
        return out, res
    return out


# revision 21
# speedup vs baseline: 1.0622x; 1.0622x over previous
"""Trainium2 Bass kernel for nn_MultiHeadAttention_2250562863251.

Key algebraic insight: the reference einsum 'mbhi,nbhj->mnbh' contracts i and j
independently, so scores[m,n,b,h] = (sum_i q[m,b,h,i]) * (sum_j k[n,b,h,j]) —
a rank-1 outer product of per-head row-sums. Full Q/K projections are never
needed; only queries @ (per-head-summed Wq) [E,H], computed on host (tiny).

Sharding: 8 cores = 2 (batch) x 4 (head-groups of 4 heads).

Device per core (batch bi, heads hg*4..hg*4+3):
  - v-proj:  v = values_b @ WvL.T   (PE, fp32r, via host-transposed valuesT)
  - scores:  scT[n,m] = qs_bcast[m]*c'_n + beta_bcast[m]   (DVE stt, fp32)
             + tri additive causal mask on diagonal blocks
  - exp:     eT = exp(scT + d_n)    (ACT, per-partition bias, bf16 out)
    where beta_m = -(qs_m * suffix-extreme(c)) = -rowmax (host), so the
    softmax max-subtraction is folded into the score build, and d_n = -1000
    padding mask folds into the ACT bias.
  - attn:    pooled[m, 65] accumulates eT.T @ [v_h | ones] over n-chunks
             (bf16 matmul; ones column yields the softmax denominator)
  - divide:  pooled[:, :64] * recip(pooled[:, 64])  per head
  - out-proj: outT[o, m] = WoLT.T @ pooledT (PE transpose + fp32r matmul)
Host assembles: out[m,b,:] = sum_hg outT.T + bo + bv @ Wo.T, with exact
recompute of (rare/absent) degenerate rows where rowmax <= -990.
"""
import sys

for _p in ("/opt/trn_rl_repo", "/root/.axon_site/_ro/trn_rl_repo"):
    if _p not in sys.path:
        sys.path.append(_p)

import numpy as np
import ml_dtypes

import concourse.bass as bass
import concourse.mybir as mybir
import concourse.tile as tile
from concourse import bacc
from concourse.bass_utils import run_bass_kernel_spmd
from concourse.masks import make_identity

# Problem shapes (hardcoded per contract)
M = 2048   # query positions
N = 2048   # key positions
B = 2
E = 1024
H = 16
DH = 64        # head dim
HL = 4         # heads per core
KL = HL * DH   # 256 local pooled dims
NEG = -1000.0
P = 128
NK = N // P    # 16 n-chunks
T = 4          # m-tiles of 512
MT = 512
NCORES = 8

f32 = mybir.dt.float32
f32r = mybir.dt.float32r
bf16 = mybir.dt.bfloat16

_CACHE = {}


def _build_program():
    if "nc" in _CACHE:
        return _CACHE["nc"]
    nc = bacc.Bacc("TRN2", target_bir_lowering=False, debug=False,
                   num_devices=NCORES)

    vt_d = nc.declare_dram_parameter("vt", [E, N], bf16, isOutput=False)
    wvlt_d = nc.declare_dram_parameter("wvlt", [E, KL], bf16, isOutput=False)
    wolt_d = nc.declare_dram_parameter("wolt", [KL, E], bf16, isOutput=False)
    qsl_d = nc.declare_dram_parameter("qsl", [HL, M], f32, isOutput=False)
    betal_d = nc.declare_dram_parameter("betal", [HL, M], bf16, isOutput=False)
    cd_d = nc.declare_dram_parameter("cd", [N, HL + 1], f32, isOutput=False)
    tri_d = nc.declare_dram_parameter("tri", [4 * P, MT], f32, isOutput=False)
    # blocked output: [ob, t, 128, 512] -> host reassembles to [E, M]
    outp_d = nc.declare_dram_parameter("outp", [E // P, T, P, MT], f32,
                                       isOutput=True)

    with tile.TileContext(nc) as tc:
        with (
            tc.tile_pool(name="const", bufs=1) as const,
            tc.tile_pool(name="work", bufs=3) as work,
            tc.tile_pool(name="et_pool", bufs=4) as et_pool,
            tc.tile_pool(name="ptn", bufs=4) as ptn,
            tc.tile_pool(name="small", bufs=4) as small,
            tc.tile_pool(name="opool", bufs=3) as opool,
            tc.tile_pool(name="dpool", bufs=4, space="DRAM") as dpool,
            tc.tile_pool(name="ps_v", bufs=2, space="PSUM") as ps_v,
            tc.tile_pool(name="ps_pool", bufs=2, space="PSUM") as ps_pool,
            tc.tile_pool(name="ps_o", bufs=2, space="PSUM") as ps_o,
        ):
            # ---- resident constants (bulk DMAs, few descriptors) ----
            wvlt_sb = const.tile([P, E // P, KL], bf16)
            nc.sync.dma_start(wvlt_sb[:], wvlt_d.rearrange("(ek p) d -> p ek d", p=P))
            wolt_sb = const.tile([DH, HL, E], bf16)
            nc.sync.dma_start(wolt_sb[:], wolt_d.rearrange("(kb p) o -> p kb o", p=DH))
            cd_sb = const.tile([P, NK, HL + 1], f32)
            nc.sync.dma_start(cd_sb[:], cd_d.rearrange("(k p) f -> p k f", p=P))
            tri_sb = const.tile([P, 4, MT], f32)
            nc.sync.dma_start(tri_sb[:], tri_d.rearrange("(pos p) m -> p pos m", p=P))
            vt_sb = const.tile([P, E // P, N], bf16)
            nc.sync.dma_start(vt_sb[:], vt_d.rearrange("(ek p) n -> p ek n", p=P))
            qsb = const.tile([P, HL, M], f32)
            nc.sync.dma_start(qsb[:], qsl_d[None, :, :].to_broadcast([P, HL, M]))
            bb = const.tile([P, HL, M], bf16)
            nc.sync.dma_start(bb[:], betal_d[None, :, :].to_broadcast([P, HL, M]))

            # v_sb[:, k, h*65 : h*65+64] = v for head h, chunk k; col 64 = 1.0
            v_sb = const.tile([P, NK, HL * (DH + 1)], bf16)
            nc.vector.memset(v_sb[:], 1.0)

            # ---- stage 1: v projection ----
            for k in range(NK):
                vps = ps_v.tile([P, KL], f32)
                for ek in range(E // P):
                    nc.tensor.matmul(
                        vps[:],
                        vt_sb[:, ek, k * P:(k + 1) * P],
                        wvlt_sb[:, ek, :],
                        start=(ek == 0),
                        stop=(ek == E // P - 1),
                    )
                nc.any.tensor_copy(
                    out=v_sb[:, k].rearrange("p (h x) -> p h x", x=DH + 1)[:, :, 0:DH],
                    in_=vps.rearrange("p (h x) -> p h x", x=DH),
                )

            # ---- stage 2: scores / softmax / attention ----
            # k runs DESCENDING so the first (widest) chunk opens the psum
            # accumulation at full width; diagonal chunks only compute the
            # live column range W = (pos+1)*128.
            pTn_all = {}
            for t in range(T):
                pTn = ptn.tile([DH, HL, MT], bf16)
                pTn_all[t] = pTn
                for h in range(HL):
                    pool_ps = ps_pool.tile([DH + 1, MT], f32)
                    for k in range(NK - 1, 4 * t - 1, -1):
                        pos = k - 4 * t
                        W = MT if pos >= 4 else (pos + 1) * P
                        sc = work.tile([P, MT], f32, tag="sc")
                        nc.vector.scalar_tensor_tensor(
                            out=sc[:, 0:W],
                            in0=qsb[:, h, t * MT:t * MT + W],
                            scalar=cd_sb[:, k, h:h + 1],
                            in1=bb[:, h, t * MT:t * MT + W],
                            op0=mybir.AluOpType.mult,
                            op1=mybir.AluOpType.add,
                        )
                        if pos < 4:
                            nc.gpsimd.tensor_add(
                                out=sc[:, 0:W], in0=sc[:, 0:W],
                                in1=tri_sb[:, pos, 0:W],
                            )
                        et = et_pool.tile([P, MT], bf16, tag="et")
                        nc.scalar.activation(
                            et[:, 0:W], sc[:, 0:W],
                            mybir.ActivationFunctionType.Exp,
                            bias=cd_sb[:, k, HL:HL + 1],
                        )
                        nc.tensor.matmul(
                            pool_ps[:, 0:W],
                            v_sb[:, k, h * (DH + 1):(h + 1) * (DH + 1)],
                            et[:, 0:W],
                            start=(k == NK - 1),
                            stop=(k == 4 * t),
                        )
                    # 1/rowsum = exp(-ln(rowsum)) on ACT (reads psum
                    # directly), DRAM-bounced to broadcast across partitions
                    rs = small.tile([DH + 1, MT], f32, tag="rs")
                    nc.scalar.activation(rs[DH:DH + 1, :], pool_ps[DH:DH + 1, :],
                                         mybir.ActivationFunctionType.Ln)
                    rs2 = small.tile([DH + 1, MT], f32, tag="rs2")
                    nc.scalar.activation(rs2[DH:DH + 1, :], rs[DH:DH + 1, :],
                                         mybir.ActivationFunctionType.Exp,
                                         scale=-1.0)
                    rd = dpool.tile([1, MT], f32, tag="rd")
                    nc.sync.dma_start(rd[:], rs2[DH:DH + 1, :])
                    rsb = small.tile([DH, MT], f32, tag="rsb")
                    nc.sync.dma_start(
                        rsb[:], rd[0][None, :].to_broadcast([DH, MT])
                    )
                    nc.vector.tensor_mul(
                        out=pTn[:, h, :],
                        in0=pool_ps[0:DH, :],
                        in1=rsb[:],
                    )

            # ---- out-projection: outT[o, m] = sum_h WoLT_h.T @ pTn_h ----
            for ob in range(E // P):
                for t in range(T):
                    ops = ps_o.tile([P, MT], f32)
                    for kb in range(HL):
                        nc.tensor.matmul(
                            ops[:],
                            wolt_sb[0:DH, kb, ob * P:(ob + 1) * P],
                            pTn_all[t][:, kb, :],
                            start=(kb == 0),
                            stop=(kb == HL - 1),
                        )
                    osb = opool.tile([P, MT], f32, tag="osb")
                    nc.scalar.copy(osb[:], ops[:])
                    nc.sync.dma_start(outp_d[ob, t], osb[:])

    nc.compile()
    _CACHE["nc"] = nc
    return nc


def _host_prep(queries, keys, values, Wq, bq, Wk, bk, Wv, bv, Wo, bo, in_mask):
    """Host-side prep. Returns (in_maps, fixup, extras)."""
    qs = np.einsum("mbe,he->mbh", queries, Wq.reshape(H, DH, E).sum(1),
                   dtype=np.float32) + bq.reshape(H, DH).sum(1)
    ks = np.einsum("nbe,he->nbh", keys, Wk.reshape(H, DH, E).sum(1),
                   dtype=np.float32) + bk.reshape(H, DH).sum(1)

    mask3 = in_mask[:, :, None]
    cp = np.where(mask3, 0.0, ks).astype(np.float32)          # [n, b, H]
    d = np.where(in_mask, NEG, 0.0).astype(np.float32)        # [n, b]

    cmax = np.where(mask3, -np.inf, ks)
    cmax = np.maximum.accumulate(cmax[::-1], axis=0)[::-1]    # suffix max, n>=m
    cmin = np.where(mask3, np.inf, ks)
    cmin = np.minimum.accumulate(cmin[::-1], axis=0)[::-1]
    nonempty = np.maximum.accumulate((~in_mask)[::-1], axis=0)[::-1]  # [n, b]

    with np.errstate(invalid="ignore"):
        A = np.where(qs >= 0, qs * cmax, qs * cmin)           # [m, b, H]
    A = np.where(nonempty[:, :, None], A, -np.inf)
    fixup_rows = np.any(~(A > -990.0), axis=2)                # [m, b] (nan-safe)
    beta = np.where(np.isfinite(A), -A, 1e4)
    beta = np.where(np.any(~(A > -990.0), axis=2)[:, :, None], -1e4, beta)
    beta = beta.astype(np.float32)

    in_maps = []
    vt_by_b = [np.ascontiguousarray(values[:, bi, :].T).astype(ml_dtypes.bfloat16)
               for bi in range(B)]
    tri = np.zeros((4 * P, MT), np.float32)
    for pos in range(4):
        nr = np.arange(P)[:, None] + 128 * pos
        mr = np.arange(MT)[None, :]
        tri[pos * P:(pos + 1) * P] = np.where(nr < mr, -4000.0, 0.0)

    for c in range(NCORES):
        bi, hg = c // 4, c % 4
        lh = slice(hg * HL, (hg + 1) * HL)
        ds = slice(hg * KL, (hg + 1) * KL)
        in_maps.append({
            "vt": vt_by_b[bi],
            "wvlt": np.ascontiguousarray(Wv[ds, :].T).astype(ml_dtypes.bfloat16),
            "wolt": np.ascontiguousarray(Wo[:, ds].T).astype(ml_dtypes.bfloat16),
            "qsl": np.ascontiguousarray(qs[:, bi, lh].T),
            "betal": np.ascontiguousarray(beta[:, bi, lh].T).astype(ml_dtypes.bfloat16),
            "cd": np.ascontiguousarray(
                np.concatenate([cp[:, bi, lh], d[:, bi:bi + 1]], axis=1)),
            "tri": tri,
        })
    return in_maps, fixup_rows, (qs, ks)


def _fixup_row(out, m, bi, qs, ks, values, Wv, bv, Wo, bo, in_mask):
    """Exact numpy recompute of one output row (degenerate / extreme rows)."""
    pot = qs[m, bi, :][None, :] * ks[:, bi, :]                # [n, H]
    pot = np.where(in_mask[:, bi][:, None], NEG, pot)
    causal = np.arange(N) < m                                 # mask n < m
    pot = np.where(causal[:, None], NEG, pot)
    pot = pot - pot.max(axis=0, keepdims=True)
    w = np.exp(pot)
    w = w / w.sum(axis=0, keepdims=True)                      # [n, H]
    v = (values[:, bi, :] @ Wv.T + bv).reshape(N, H, DH)
    pooled = np.einsum("nh,nhd->hd", w, v).reshape(E)
    out[m, bi, :] = pooled @ Wo.T + bo


def kernel(queries, keys, values, Wq, bq, Wk, bk, Wv, bv, Wo, bo, in_mask,
           _trace=False):
    args = (queries, keys, values, Wq, bq, Wk, bk, Wv, bv, Wo, bo)
    args = tuple(np.asarray(a, np.float32) for a in args)
    in_mask = np.asarray(in_mask, bool)
    (queries, keys, values, Wq, bq, Wk, bk, Wv, bv, Wo, bo) = args

    nc = _build_program()
    in_maps, fixup_rows, (qs, ks) = _host_prep(
        queries, keys, values, Wq, bq, Wk, bk, Wv, bv, Wo, bo, in_mask)

    res = run_bass_kernel_spmd(nc, in_maps, list(range(NCORES)), trace=_trace)
    results = res.results

    out = np.zeros((M, B, E), np.float32)
    for c in range(NCORES):
        bi = c // 4
        blk = np.asarray(results[c]["outp"], np.float32)   # [8, 4, 128, 512]
        outT = blk.transpose(0, 2, 1, 3).reshape(E, M)
        out[:, bi, :] += outT.T
    out += (bo + bv @ Wo.T)[None, None, :]

    for m, bi in zip(*np.nonzero(fixup_rows)):
        _fixup_row(out, m, bi, qs, ks, values, Wv, bv, Wo, bo, in_mask)

    if _trace:
        return out, res
    return out


# revision 24
# speedup vs baseline: 1.2112x; 1.1402x over previous
"""Trainium2 Bass kernel for nn_MultiHeadAttention_2250562863251.

Key algebraic insight: the reference einsum 'mbhi,nbhj->mnbh' contracts i and j
independently, so scores[m,n,b,h] = (sum_i q[m,b,h,i]) * (sum_j k[n,b,h,j]) —
a rank-1 outer product of per-head row-sums. Full Q/K projections are never
needed; only queries @ (per-head-summed Wq) [E,H], computed on host (tiny).

Sharding: 8 cores = 2 (batch) x 4 (head-groups of 4 heads).

Device per core (batch bi, heads hg*4..hg*4+3):
  - v-proj:  v = values_b @ WvL.T   (PE, fp32r, via host-transposed valuesT)
  - scores:  scT[n,m] = qs_bcast[m]*c'_n + beta_bcast[m]   (DVE stt, fp32)
             + tri additive causal mask on diagonal blocks
  - exp:     eT = exp(scT + d_n)    (ACT, per-partition bias, bf16 out)
    where beta_m = -(qs_m * suffix-extreme(c)) = -rowmax (host), so the
    softmax max-subtraction is folded into the score build, and d_n = -1000
    padding mask folds into the ACT bias.
  - attn:    pooled[m, 65] accumulates eT.T @ [v_h | ones] over n-chunks
             (bf16 matmul; ones column yields the softmax denominator)
  - divide:  pooled[:, :64] * recip(pooled[:, 64])  per head
  - out-proj: outT[o, m] = WoLT.T @ pooledT (PE transpose + fp32r matmul)
Host assembles: out[m,b,:] = sum_hg outT.T + bo + bv @ Wo.T, with exact
recompute of (rare/absent) degenerate rows where rowmax <= -990.
"""
import sys

for _p in ("/opt/trn_rl_repo", "/root/.axon_site/_ro/trn_rl_repo"):
    if _p not in sys.path:
        sys.path.append(_p)

import numpy as np
import ml_dtypes

import concourse.bass as bass
import concourse.mybir as mybir
import concourse.tile as tile
from concourse import bacc
from concourse.bass_utils import run_bass_kernel_spmd
from concourse.masks import make_identity

# Problem shapes (hardcoded per contract)
M = 2048   # query positions
N = 2048   # key positions
B = 2
E = 1024
H = 16
DH = 64        # head dim
HL = 4         # heads per core
KL = HL * DH   # 256 local pooled dims
NEG = -1000.0
P = 128
NK = N // P    # 16 n-chunks
T = 4          # m-tiles of 512
MT = 512
NCORES = 8

f32 = mybir.dt.float32
f32r = mybir.dt.float32r
bf16 = mybir.dt.bfloat16

_CACHE = {}


def _build_program():
    if "nc" in _CACHE:
        return _CACHE["nc"]
    nc = bacc.Bacc("TRN2", target_bir_lowering=False, debug=False,
                   num_devices=NCORES)

    vt_d = nc.declare_dram_parameter("vt", [P, 4, (E // P) * MT], bf16, isOutput=False)
    wvlt_d = nc.declare_dram_parameter("wvlt", [P, (E // P) * KL], bf16, isOutput=False)
    wolt_d = nc.declare_dram_parameter("wolt", [KL, E], bf16, isOutput=False)
    qsl_d = nc.declare_dram_parameter("qsl", [HL, M], f32, isOutput=False)
    betal_d = nc.declare_dram_parameter("betal", [HL, M], bf16, isOutput=False)
    cd_d = nc.declare_dram_parameter("cd", [P, NK * (HL + 1)], f32, isOutput=False)
    tri_d = nc.declare_dram_parameter("tri", [P, 4 * MT], f32, isOutput=False)
    # blocked output: [ob, t, 128, 512] -> host reassembles to [E, M]
    outp_d = nc.declare_dram_parameter("outp", [E // P, T, P, MT], f32,
                                       isOutput=True)

    with tile.TileContext(nc) as tc:
        with (
            tc.tile_pool(name="const", bufs=1) as const,
            tc.tile_pool(name="work", bufs=3) as work,
            tc.tile_pool(name="vstream", bufs=2) as vstream,
            tc.tile_pool(name="rspool", bufs=2) as rspool,
            tc.tile_pool(name="et_pool", bufs=4) as et_pool,
            tc.tile_pool(name="ptn", bufs=4) as ptn,
            tc.tile_pool(name="small", bufs=4) as small,
            tc.tile_pool(name="opool", bufs=3) as opool,
            tc.tile_pool(name="dpool", bufs=4, space="DRAM") as dpool,
            tc.tile_pool(name="ps_v", bufs=2, space="PSUM") as ps_v,
            tc.tile_pool(name="ps_pool", bufs=1, space="PSUM") as ps_pool,
        ):
            # ---- resident constants (bulk DMAs, few descriptors) ----
            wvlt_sb = const.tile([P, E // P, KL], bf16)
            nc.sync.dma_start(wvlt_sb[:], wvlt_d.rearrange("p (ek d) -> p ek d", ek=E // P))
            wolt_sb = const.tile([DH, HL, E], bf16)
            nc.sync.dma_start(wolt_sb[:], wolt_d.rearrange("(kb p) o -> p kb o", p=DH))
            cd_sb = const.tile([P, NK, HL + 1], f32)
            nc.sync.dma_start(cd_sb[:], cd_d.rearrange("p (k f) -> p k f", k=NK))
            tri_sb = const.tile([P, 4, MT], f32)
            nc.sync.dma_start(tri_sb[:], tri_d.rearrange("p (pos m) -> p pos m", pos=4))

            qsb = const.tile([P, HL, M], f32)
            nc.sync.dma_start(qsb[:], qsl_d[None, :, :].to_broadcast([P, HL, M]))
            bb = const.tile([P, HL, M], bf16)
            nc.sync.dma_start(bb[:], betal_d[None, :, :].to_broadcast([P, HL, M]))

            # v_sb[:, k, h*65 : h*65+64] = v for head h, chunk k; col 64 = 1.0
            v_sb = const.tile([P, NK, HL * (DH + 1)], bf16)
            nc.vector.memset(v_sb[:], 1.0)

            # ---- stage 1: v projection (vt streamed per n-quarter) ----
            for q in range(4):
                vt_sb = vstream.tile([P, E // P, MT], bf16, tag="vt")
                nc.sync.dma_start(
                    vt_sb[:], vt_d[:, q].rearrange("p (ek n) -> p ek n", ek=E // P))
                for nk_r in range(4):
                    k = q * 4 + nk_r
                    vps = ps_v.tile([P, KL], f32, tag="vps")
                    for ek in range(E // P):
                        nc.tensor.matmul(
                            vps[:],
                            vt_sb[:, ek, nk_r * P:(nk_r + 1) * P],
                            wvlt_sb[:, ek, :],
                            start=(ek == 0),
                            stop=(ek == E // P - 1),
                        )
                    nc.any.tensor_copy(
                        out=v_sb[:, k].rearrange("p (h x) -> p h x", x=DH + 1)[:, :, 0:DH],
                        in_=vps.rearrange("p (h x) -> p h x", x=DH),
                    )

            # ---- stage 2: scores / softmax / attention / out-proj ----
            # k runs DESCENDING so the first (widest) chunk opens the psum
            # accumulation at full width; diagonal chunks only compute the
            # live column range W = (pos+1)*128. Reciprocals are grouped per
            # m-tile to minimize ACT table reloads (Exp->Ln->Exp switches).
            for t in range(T):
                pools = []
                for h in range(HL):
                    pool_ps = ps_pool.tile([DH + 1, MT], f32, tag=f"pool{h}")
                    pools.append(pool_ps)
                    for k in range(NK - 1, 4 * t - 1, -1):
                        pos = k - 4 * t
                        W = MT if pos >= 4 else (pos + 1) * P
                        sc = work.tile([P, MT], f32, tag="sc")
                        nc.vector.scalar_tensor_tensor(
                            out=sc[:, 0:W],
                            in0=qsb[:, h, t * MT:t * MT + W],
                            scalar=cd_sb[:, k, h:h + 1],
                            in1=bb[:, h, t * MT:t * MT + W],
                            op0=mybir.AluOpType.mult,
                            op1=mybir.AluOpType.add,
                        )
                        if pos < 4:
                            nc.gpsimd.tensor_add(
                                out=sc[:, 0:W], in0=sc[:, 0:W],
                                in1=tri_sb[:, pos, 0:W],
                            )
                        et = et_pool.tile([P, MT], bf16, tag="et")
                        nc.scalar.activation(
                            et[:, 0:W], sc[:, 0:W],
                            mybir.ActivationFunctionType.Exp,
                            bias=cd_sb[:, k, HL:HL + 1],
                        )
                        nc.tensor.matmul(
                            pool_ps[:, 0:W],
                            v_sb[:, k, h * (DH + 1):(h + 1) * (DH + 1)],
                            et[:, 0:W],
                            start=(k == NK - 1),
                            stop=(k == 4 * t),
                        )
                # 1/rowsum = exp(-ln(rowsum)) on ACT, grouped: Ln x4 then
                # Exp(-1) x4 (2 table switches per tile instead of 8)
                rs = rspool.tile([DH + 1, HL, MT], f32, tag="rs")
                for h in range(HL):
                    nc.scalar.activation(rs[DH:DH + 1, h], pools[h][DH:DH + 1, :],
                                         mybir.ActivationFunctionType.Ln)
                rs2 = rspool.tile([DH + 1, HL, MT], f32, tag="rs2")
                for h in range(HL):
                    nc.scalar.activation(rs2[DH:DH + 1, h], rs[DH:DH + 1, h],
                                         mybir.ActivationFunctionType.Exp,
                                         scale=-1.0)
                pTn = ptn.tile([DH, HL, MT], bf16)
                for h in range(HL):
                    rd = dpool.tile([1, MT], f32, tag="rd")
                    nc.sync.dma_start(rd[:], rs2[DH:DH + 1, h])
                    rsb = small.tile([DH, MT], f32, tag="rsb")
                    nc.sync.dma_start(
                        rsb[:], rd[0][None, :].to_broadcast([DH, MT])
                    )
                    nc.vector.tensor_mul(
                        out=pTn[:, h, :],
                        in0=pools[h][0:DH, :],
                        in1=rsb[:],
                    )

                # out-projection for this m-tile (overlaps next tile's scores)
                for ob in range(E // P):
                    ops = ps_v.tile([P, MT], f32, tag="ops")
                    for kb in range(HL):
                        nc.tensor.matmul(
                            ops[:],
                            wolt_sb[0:DH, kb, ob * P:(ob + 1) * P],
                            pTn[:, kb, :],
                            start=(kb == 0),
                            stop=(kb == HL - 1),
                        )
                    osb = opool.tile([P, MT], f32, tag="osb")
                    if ob % 2 == 0:
                        nc.vector.tensor_copy(out=osb[:], in_=ops[:])
                    else:
                        nc.scalar.copy(osb[:], ops[:])
                    nc.sync.dma_start(outp_d[ob, t], osb[:])

    nc.compile()
    _CACHE["nc"] = nc
    return nc


def _host_prep(queries, keys, values, Wq, bq, Wk, bk, Wv, bv, Wo, bo, in_mask):
    """Host-side prep. Returns (in_maps, fixup, extras)."""
    qs = np.einsum("mbe,he->mbh", queries, Wq.reshape(H, DH, E).sum(1),
                   dtype=np.float32) + bq.reshape(H, DH).sum(1)
    ks = np.einsum("nbe,he->nbh", keys, Wk.reshape(H, DH, E).sum(1),
                   dtype=np.float32) + bk.reshape(H, DH).sum(1)

    mask3 = in_mask[:, :, None]
    cp = np.where(mask3, 0.0, ks).astype(np.float32)          # [n, b, H]
    d = np.where(in_mask, NEG, 0.0).astype(np.float32)        # [n, b]

    cmax = np.where(mask3, -np.inf, ks)
    cmax = np.maximum.accumulate(cmax[::-1], axis=0)[::-1]    # suffix max, n>=m
    cmin = np.where(mask3, np.inf, ks)
    cmin = np.minimum.accumulate(cmin[::-1], axis=0)[::-1]
    nonempty = np.maximum.accumulate((~in_mask)[::-1], axis=0)[::-1]  # [n, b]

    with np.errstate(invalid="ignore"):
        A = np.where(qs >= 0, qs * cmax, qs * cmin)           # [m, b, H]
    A = np.where(nonempty[:, :, None], A, -np.inf)
    fixup_rows = np.any(~(A > -990.0), axis=2)                # [m, b] (nan-safe)
    beta = np.where(np.isfinite(A), -A, 1e4)
    beta = np.where(np.any(~(A > -990.0), axis=2)[:, :, None], -1e4, beta)
    beta = beta.astype(np.float32)

    in_maps = []
    def pmajor(a, p=P):
        """[X*p, Y] -> [p, X*Y]: partition-major packing for 1-run-per-
        partition DMA loads matching 'p (x y) -> p x y' device views."""
        X = a.shape[0] // p
        return np.ascontiguousarray(
            a.reshape(X, p, a.shape[1]).transpose(1, 0, 2).reshape(p, -1))

    def pack_vt(vT):
        # [E, N] -> [P, 4, (E//P)*MT]: quarter-major, then ek-major
        a = vT.reshape(E // P, P, 4, MT)          # [ek, p, q, mt]
        return np.ascontiguousarray(
            a.transpose(1, 2, 0, 3).reshape(P, 4, (E // P) * MT))

    vt_by_b = [pack_vt(values[:, bi, :].T.astype(ml_dtypes.bfloat16))
               for bi in range(B)]
    tri = np.zeros((4 * P, MT), np.float32)
    for pos in range(4):
        nr = np.arange(P)[:, None] + 128 * pos
        mr = np.arange(MT)[None, :]
        tri[pos * P:(pos + 1) * P] = np.where(nr < mr, -4000.0, 0.0)
    tri_pm = np.ascontiguousarray(
        tri.reshape(4, P, MT).transpose(1, 0, 2).reshape(P, -1))

    for c in range(NCORES):
        bi, hg = c // 4, c % 4
        lh = slice(hg * HL, (hg + 1) * HL)
        ds = slice(hg * KL, (hg + 1) * KL)
        in_maps.append({
            "vt": vt_by_b[bi],
            "wvlt": pmajor(Wv[ds, :].T.astype(ml_dtypes.bfloat16)),
            "wolt": np.ascontiguousarray(Wo[:, ds].T).astype(ml_dtypes.bfloat16),
            "qsl": np.ascontiguousarray(qs[:, bi, lh].T),
            "betal": np.ascontiguousarray(beta[:, bi, lh].T).astype(ml_dtypes.bfloat16),
            "cd": pmajor(np.ascontiguousarray(
                np.concatenate([cp[:, bi, lh], d[:, bi:bi + 1]], axis=1))),
            "tri": tri_pm,
        })
    return in_maps, fixup_rows, (qs, ks)


def _fixup_row(out, m, bi, qs, ks, values, Wv, bv, Wo, bo, in_mask):
    """Exact numpy recompute of one output row (degenerate / extreme rows)."""
    pot = qs[m, bi, :][None, :] * ks[:, bi, :]                # [n, H]
    pot = np.where(in_mask[:, bi][:, None], NEG, pot)
    causal = np.arange(N) < m                                 # mask n < m
    pot = np.where(causal[:, None], NEG, pot)
    pot = pot - pot.max(axis=0, keepdims=True)
    w = np.exp(pot)
    w = w / w.sum(axis=0, keepdims=True)                      # [n, H]
    v = (values[:, bi, :] @ Wv.T + bv).reshape(N, H, DH)
    pooled = np.einsum("nh,nhd->hd", w, v).reshape(E)
    out[m, bi, :] = pooled @ Wo.T + bo


def kernel(queries, keys, values, Wq, bq, Wk, bk, Wv, bv, Wo, bo, in_mask,
           _trace=False):
    args = (queries, keys, values, Wq, bq, Wk, bk, Wv, bv, Wo, bo)
    args = tuple(np.asarray(a, np.float32) for a in args)
    in_mask = np.asarray(in_mask, bool)
    (queries, keys, values, Wq, bq, Wk, bk, Wv, bv, Wo, bo) = args

    nc = _build_program()
    in_maps, fixup_rows, (qs, ks) = _host_prep(
        queries, keys, values, Wq, bq, Wk, bk, Wv, bv, Wo, bo, in_mask)

    res = run_bass_kernel_spmd(nc, in_maps, list(range(NCORES)), trace=_trace)
    results = res.results

    out = np.zeros((M, B, E), np.float32)
    for c in range(NCORES):
        bi = c // 4
        blk = np.asarray(results[c]["outp"], np.float32)   # [8, 4, 128, 512]
        outT = blk.transpose(0, 2, 1, 3).reshape(E, M)
        out[:, bi, :] += outT.T
    out += (bo + bv @ Wo.T)[None, None, :]

    for m, bi in zip(*np.nonzero(fixup_rows)):
        _fixup_row(out, m, bi, qs, ks, values, Wv, bv, Wo, bo, in_mask)

    if _trace:
        return out, res
    return out
